# revision 5
# baseline (speedup 1.0000x reference)
"""Trainium2 Bass kernel for nn_Attention_17635135717804.

Dense transformer attention block (LeViT-style):
  qkv = BN(x @ Wqkv.T); per-head attention with gathered relative-position
  bias; softmax; o = attn @ v; y = BN(hardswish(o) @ Wproj.T).

Data-parallel over batch across 8 NeuronCores (16 batches/core). BN is
folded into weights host-side; softmax SCALE folded into Wq; x is
pre-transposed to xT[dim, n] bf16 on host (kills the on-device PE
transposes and their DVE evictions).

On real TRN2 the PE runs bf16 matmuls at ~0.17 ns/column (2x the cost
model), so this problem is bound by PSUM-eviction bandwidth on DVE+ACT,
not by the matmuls. The kernel is organized around that:
  - softmax normalize runs on DVE (bf16 2x mode) with reciprocals
    grouped 4 heads per op.
  - hardswish is 2 DVE passes (tensor_scalar + scalar_tensor_tensor)
    with the 1/6 folded into Wproj.
  - PSUM evictions are split across DVE and ACT to balance both queues
    (v + attn-transpose evicts on DVE; exp, qk and proj evicts on ACT;
    attn@v evicts split).
  - the batch loop is a 3-stage software pipeline: hardswish+proj of
    batch b-2 and qkv of batch b are emitted before attention of b-1,
    so the vector engines always have eviction work queued while the PE
    runs the next batch's matmuls.
"""

import numpy as np
import ml_dtypes

RES = 14
DIM = 512
KD = 64
H = 16
D = 256
DH = H * D            # 4096
HID = DH + 2 * H * KD  # 6144
B = 128
N = RES * RES         # 196
EPS = 1e-5
SCALE = KD ** -0.5

NCORES = 8
BPC = B // NCORES     # 16 batches per core
P = 128
NT1 = N - P           # 68: second token tile
NKT = DIM // P        # 4 k-tiles over input dim
QKF = 2 * H * KD      # 2048 qk features
BF16 = ml_dtypes.bfloat16

_PROGRAM_CACHE = {}


def _build_program(repeat=1):
    """Build the per-core Bass/Tile program (identical on all 8 cores).

    repeat>1 re-runs the whole batch loop (same data) for slope-based
    timing: T(R) - T(1) = (R-1) * kernel_time."""
    if repeat in _PROGRAM_CACHE:
        return _PROGRAM_CACHE[repeat]

    import concourse.bass as bass
    import concourse.mybir as mybir
    import concourse.tile as tile
    from concourse.masks import make_identity

    f32 = mybir.dt.float32
    bf16 = mybir.dt.bfloat16
    AF = mybir.ActivationFunctionType
    OP = mybir.AluOpType

    nc = bass.Bass("TRN2", target_bir_lowering=False, debug=False)

    xt_d = nc.dram_tensor("xt", [BPC, P, NKT, N], bf16, kind="ExternalInput").ap()
    wqk_d = nc.dram_tensor("wqk", [P, NKT, QKF], bf16, kind="ExternalInput").ap()
    wv_d = nc.dram_tensor("wv", [P, NKT, DH], bf16, kind="ExternalInput").ap()
    wp_d = nc.dram_tensor("wp", [P, DH // P, DIM], bf16, kind="ExternalInput").ap()
    bias_d = nc.dram_tensor("bias", [P, H, 2 * N], bf16, kind="ExternalInput").ap()
    c1qk_d = nc.dram_tensor("c1qk", [P, H], f32, kind="ExternalInput").ap()
    c1v_d = nc.dram_tensor("c1v", [P, DH // P], f32, kind="ExternalInput").ap()
    c2_d = nc.dram_tensor("c2", [P, DIM // P], f32, kind="ExternalInput").ap()
    y_d = nc.dram_tensor("y", [BPC, P, DIM // P, N], f32, kind="ExternalOutput").ap()

    from contextlib import ExitStack

    with tile.TileContext(nc) as tc:
        with ExitStack() as ctx:
            pool_ = lambda name, bufs, **kw: ctx.enter_context(
                tc.tile_pool(name=name, bufs=bufs, **kw)
            )
            singles = pool_("singles", 1)
            xTpool = pool_("xTpool", 3)
            qkpool = pool_("qkpool", 2)
            vpool = pool_("vpool", 2)
            epool = pool_("epool", 8)
            apool = pool_("apool", 3)
            sumpool = pool_("sumpool", 3)
            aTpool = pool_("aTpool", 4)
            zpool = pool_("zpool", 2)
            upool = pool_("upool", 2)
            yTpool = pool_("yTpool", 2)
            # PSUM: pmm (qk [128,196] / AV po [128,392]) + pv + ps + paT
            pmm = pool_("pmm", 2, space="PSUM")
            pv_pool = pool_("pv", 2, space="PSUM")
            ps_pool = pool_("ps", 2, space="PSUM")
            paT_pool = pool_("paT", 2, space="PSUM")

            # resident constants
            c1qk = singles.tile([P, H], f32)
            nc.scalar.dma_start(out=c1qk, in_=c1qk_d)
            c1v = singles.tile([P, DH // P], f32)
            nc.scalar.dma_start(out=c1v, in_=c1v_d)
            c2 = singles.tile([P, DIM // P], f32)
            nc.scalar.dma_start(out=c2, in_=c2_d)
            wqk = singles.tile([P, NKT, QKF], bf16)
            nc.scalar.dma_start(out=wqk, in_=wqk_d)
            wv = singles.tile([P, NKT, DH], bf16)
            for wc in range(4):
                nc.scalar.dma_start(
                    out=wv[:, :, wc * (DH // 4):(wc + 1) * (DH // 4)],
                    in_=wv_d[:, :, wc * (DH // 4):(wc + 1) * (DH // 4)],
                )
            bias = singles.tile([P, H, 2 * N], bf16)
            nc.scalar.dma_start(out=bias, in_=bias_d)
            wp = singles.tile([P, DH // P, DIM], bf16)
            nc.scalar.dma_start(out=wp, in_=wp_d)
            ident_b = singles.tile([P, P], bf16)
            make_identity(nc, ident_b)

            def load_xT(b):
                xT = xTpool.tile([P, NKT, N], bf16, tag="xT")
                nc.sync.dma_start(out=xT, in_=xt_d[b])
                return xT

            def emit_qk(xT):
                # qkT[2048, n] = Wqk @ xT, + c1qk bias, -> bf16 (ACT evict)
                qkT = qkpool.tile([P, H, N], bf16, tag="qk")
                for h in range(H):
                    pq = pmm.tile([P, N], f32, tag="mm")
                    for kt in range(NKT):
                        nc.tensor.matmul(
                            pq, wqk[:, kt, h * P:(h + 1) * P], xT[:, kt, :],
                            start=(kt == 0), stop=(kt == NKT - 1),
                        )
                    nc.scalar.activation(
                        out=qkT[:, h, :], in_=pq, func=AF.Identity,
                        bias=c1qk[:, h:h + 1], scale=1.0,
                    )
                return qkT

            def emit_v(xT):
                # v[n, 4096] = xT.T @ WvT (token-major; DVE evicts, no bias)
                v_sb = vpool.tile([P, 2, DH], bf16, tag="v")
                for mt in range(2):
                    rows = P if mt == 0 else NT1
                    for ntc in range(DH // 512):
                        pvt = pv_pool.tile([P, 512], f32, tag="pv")
                        for kt in range(NKT):
                            nc.tensor.matmul(
                                pvt[:rows],
                                xT[:, kt, mt * P:mt * P + rows],
                                wv[:, kt, ntc * 512:(ntc + 1) * 512],
                                start=(kt == 0), stop=(kt == NKT - 1),
                            )
                        nc.vector.tensor_copy(
                            out=v_sb[:rows, mt, ntc * 512:(ntc + 1) * 512],
                            in_=pvt[:rows],
                        )
                return v_sb

            def attn_scores(qkT, h, sums, sidx):
                # scores packed [128, 392]: bias preload (PE), qk matmuls,
                # exp+accum (ACT) -> e_sb bf16
                qo = (h % 2) * KD
                qt, kt_i = h // 2, 8 + h // 2
                s_ps = ps_pool.tile([P, 2 * N], f32, tag="ps")
                nc.tensor.matmul(
                    s_ps, ident_b, bias[:, h, :], start=True, stop=False,
                )
                nc.tensor.matmul(
                    s_ps[:, 0:N], qkT[qo:qo + KD, qt, 0:P],
                    qkT[qo:qo + KD, kt_i, :],
                    start=False, stop=False,
                )
                nc.tensor.matmul(
                    s_ps[:NT1, N:2 * N], qkT[qo:qo + KD, qt, P:N],
                    qkT[qo:qo + KD, kt_i, :],
                    start=False, stop=True,
                )
                e_sb = epool.tile([P, 2 * N], bf16, tag="e")
                nc.scalar.activation(
                    out=e_sb[:, 0:N], in_=s_ps[:, 0:N], func=AF.Exp,
                    accum_out=sums[:, 2 * sidx:2 * sidx + 1],
                )
                nc.scalar.activation(
                    out=e_sb[:NT1, N:2 * N], in_=s_ps[:NT1, N:2 * N],
                    func=AF.Exp, accum_out=sums[:NT1, 2 * sidx + 1:2 * sidx + 2],
                )
                return e_sb

            def attn_norm(e_sb, sums, sidx):
                # a = e * (1/rowsum), DVE TS with per-partition scalar
                a_sb = apool.tile([P, 2 * N], bf16, tag="a")
                nc.vector.tensor_scalar_mul(
                    out=a_sb[:, 0:N], in0=e_sb[:, 0:N],
                    scalar1=sums[:, 2 * sidx:2 * sidx + 1],
                )
                nc.vector.tensor_scalar_mul(
                    out=a_sb[:NT1, N:2 * N], in0=e_sb[:NT1, N:2 * N],
                    scalar1=sums[:NT1, 2 * sidx + 1:2 * sidx + 2],
                )
                return a_sb

            def attn_transpose(a_sb):
                paT = paT_pool.tile([P, 2 * N], bf16, tag="paT")
                nc.tensor.transpose(paT[:, 0:P], a_sb[:, 0:P], ident_b)
                nc.tensor.transpose(
                    paT[:, P:N], a_sb[:NT1, N:N + P], ident_b[:NT1, :NT1]
                )
                nc.tensor.transpose(paT[:NT1, N:N + P], a_sb[:, P:N], ident_b)
                nc.tensor.transpose(
                    paT[:NT1, N + P:2 * N], a_sb[:NT1, N + P:2 * N],
                    ident_b[:NT1, :NT1],
                )
                aT_sb = aTpool.tile([P, 2 * N], bf16, tag="aT")
                nc.vector.tensor_copy(out=aT_sb, in_=paT)
                return aT_sb

            def attn_av(h, aT_sb, v_sb, z_sb):
                # oT[d, n] = v.T @ attnT (+c1v at evict; evicts split DVE/ACT)
                for dt in range(2):
                    col = h * 2 + dt
                    po = pmm.tile([P, N], f32, tag="mm")
                    nc.tensor.matmul(
                        po, v_sb[:, 0, col * P:(col + 1) * P],
                        aT_sb[:, 0:N], start=True, stop=False,
                    )
                    nc.tensor.matmul(
                        po, v_sb[:NT1, 1, col * P:(col + 1) * P],
                        aT_sb[:NT1, N:2 * N], start=False, stop=True,
                    )
                    if col % 3 == 0:   # ~1/3 on DVE, 2/3 on ACT
                        nc.vector.tensor_scalar_add(
                            out=z_sb[:, col, :], in0=po,
                            scalar1=c1v[:, col:col + 1],
                        )
                    else:
                        nc.scalar.activation(
                            out=z_sb[:, col, :], in_=po, func=AF.Identity,
                            bias=c1v[:, col:col + 1], scale=1.0,
                        )

            def emit_attention(qkT, v_sb):
                # 16 heads, software-pipelined:
                #   scores/exp(h) | diag+transpose(h-4) | AV(h-6)
                z_sb = zpool.tile([P, DH // P, N], bf16, tag="z")
                e_q, aT_q = {}, {}
                sums_g = {}
                for h in range(H + 6):
                    if h < H:
                        g = h // 4
                        if h % 4 == 0:
                            sums_g[g] = sumpool.tile([P, 8], f32, tag="sums", name=f"sums{g}")
                        e_q[h] = attn_scores(qkT, h, sums_g[g], h % 4)
                        if h % 4 == 3:
                            nc.vector.reciprocal(out=sums_g[g], in_=sums_g[g])
                    if 4 <= h < H + 4:
                        hh = h - 4
                        a_sb = attn_norm(e_q.pop(hh), sums_g[hh // 4], hh % 4)
                        aT_q[hh] = attn_transpose(a_sb)
                    if h >= 6:
                        hh = h - 6
                        attn_av(hh, aT_q.pop(hh), v_sb, z_sb)
                return z_sb

            def emit_hardswish(z_sb):
                # hardswish*6 (1/6 folded into Wp): u = relu(z+3) [DVE TS],
                # z = min(u, 6) * z [DVE STT]
                CH = 8
                u = upool.tile([P, CH, N], bf16, tag="u")
                for c0 in range(0, DH // P, CH):
                    zc = z_sb[:, c0:c0 + CH, :]
                    nc.vector.tensor_scalar(
                        out=u, in0=zc, scalar1=3.0, scalar2=0.0,
                        op0=OP.add, op1=OP.max,
                    )
                    nc.vector.scalar_tensor_tensor(
                        out=zc, in0=u, scalar=6.0, in1=zc,
                        op0=OP.min, op1=OP.mult,
                    )

            def emit_proj(z_sb, b):
                # yT[512, n] = (Wp/6) @ hardswish6(oT), + c2 (ACT evict)
                yT = yTpool.tile([P, DIM // P, N], f32, tag="yT")
                for mt in range(DIM // P):
                    py = pmm.tile([P, N], f32, tag="mm")
                    for kt in range(DH // P):
                        nc.tensor.matmul(
                            py, wp[:, kt, mt * P:(mt + 1) * P], z_sb[:, kt, :],
                            start=(kt == 0), stop=(kt == DH // P - 1),
                        )
                    nc.scalar.activation(
                        out=yT[:, mt, :], in_=py, func=AF.Identity,
                        bias=c2[:, mt:mt + 1], scale=1.0,
                    )
                nc.sync.dma_start(out=y_d[b], in_=yT)

            # ---- 3-stage pipelined batch loop ----
            batch_seq = list(range(BPC)) * repeat
            nb = len(batch_seq)
            xT_cur = load_xT(batch_seq[0])
            xT_next = load_xT(batch_seq[1]) if nb > 1 else None
            qkT_prev = v_prev = None     # stage of batch i-1
            z_prev2 = None               # z of batch i-2
            b_prev2 = None
            for i in range(nb + 2):
                if z_prev2 is not None:
                    emit_hardswish(z_prev2)
                    emit_proj(z_prev2, b_prev2)
                    z_prev2 = None
                if i < nb:
                    qkT_cur = emit_qk(xT_cur)
                    v_cur = emit_v(xT_cur)
                    xT_cur = xT_next
                    xT_next = load_xT(batch_seq[i + 2]) if i + 2 < nb else None
                else:
                    qkT_cur = v_cur = None
                if qkT_prev is not None:
                    z_prev2 = emit_attention(qkT_prev, v_prev)
                    b_prev2 = batch_seq[i - 1]
                qkT_prev, v_prev = qkT_cur, v_cur

    _split_matmul_waits(nc, mybir)
    _PROGRAM_CACHE[repeat] = nc
    return nc


def _split_matmul_waits(nc, mybir):
    """Walrus's per-instruction ISA structs accept only one sync wait;
    hoist extra waits onto injected single-wait NoOps on the same engine."""
    multiwait_ok = ("InstCall",)
    nid = [0]
    for f in nc.m.functions:
        for blk in f.blocks:
            insts = blk.instructions
            out = []
            changed = False
            for i in insts:
                si = i.sync_info
                if (
                    type(i).__name__ not in multiwait_ok
                    and si is not None
                    and si.on_wait
                    and len(si.on_wait) > 1
                ):
                    for w in si.on_wait[1:]:
                        nop = mybir.InstNoOp(
                            name=f"waitnop-{nid[0]}", ins=[], outs=[]
                        )
                        nid[0] += 1
                        nop.engine = i.engine
                        nop.sync_info = mybir.SyncInfo(
                            on_wait=[w], on_update=[]
                        )
                        out.append(nop)
                    i.sync_info = mybir.SyncInfo(
                        on_wait=[si.on_wait[0]],
                        on_update=list(si.on_update or []),
                    )
                    changed = True
                out.append(i)
            if changed:
                blk.instructions = out


def _prepare_inputs(inputs):
    """Fold BN into weights, reorder layouts, gather bias, transpose x;
    build per-core input maps."""
    f = lambda k: np.asarray(inputs[k], dtype=np.float32)
    x = f("x")
    w_qkv = f("w_qkv")
    g1, b1, m1, v1 = f("g1"), f("b1"), f("m1"), f("v1")
    bias_table = f("bias_table")
    w_proj = f("w_proj")
    g2, b2, m2, v2 = f("g2"), f("b2"), f("m2"), f("v2")
    bias_idxs = np.asarray(inputs["bias_idxs"])

    s1 = g1 / np.sqrt(v1 + EPS)
    c1 = b1 - m1 * s1
    W1 = w_qkv * s1[:, None]          # [HID, DIM]
    W1h = W1.reshape(H, 2 * KD + D, DIM)
    c1h = c1.reshape(H, 2 * KD + D)

    # qk features: tiles 0..7 hold q of head-pairs (pre-scaled by SCALE),
    # tiles 8..15 hold k of head-pairs; head h sits at partition (h%2)*64
    # of tile h//2 (q) and tile 8+h//2 (k) so q/k share a base partition.
    wqk_feat = np.empty((QKF, DIM), np.float32)
    c1qk = np.empty((P, H), np.float32)
    for h in range(H):
        qrow = (h // 2) * P + (h % 2) * KD
        krow = 8 * P + qrow
        wqk_feat[qrow:qrow + KD] = W1h[h, :KD] * SCALE
        wqk_feat[krow:krow + KD] = W1h[h, KD:2 * KD]
        c1qk[(h % 2) * KD:(h % 2) * KD + KD, h // 2] = c1h[h, :KD] * SCALE
        c1qk[(h % 2) * KD:(h % 2) * KD + KD, 8 + h // 2] = c1h[h, KD:2 * KD]
    # lhsT layout [dim_p, ktile, feat]
    wqk_l = np.ascontiguousarray(
        wqk_feat.T.reshape(NKT, P, QKF).transpose(1, 0, 2)
    ).astype(BF16)

    # v features (h, d) -> rhs layout [dim_p, ktile, dh]
    wv_feat = W1h[:, 2 * KD:, :].reshape(DH, DIM)
    wv_l = np.ascontiguousarray(
        wv_feat.T.reshape(NKT, P, DH).transpose(1, 0, 2)
    ).astype(BF16)
    c1v = np.ascontiguousarray(
        c1h[:, 2 * KD:].reshape(DH).reshape(DH // P, P).T
    ).astype(np.float32)

    s2 = g2 / np.sqrt(v2 + EPS)
    c2 = b2 - m2 * s2
    # hardswish computed as z*clip(z+3,0,6); fold the /6 into Wproj
    W2 = w_proj * s2[:, None] * (1.0 / 6.0)   # [DIM, DH]
    wp_l = np.ascontiguousarray(
        W2.T.reshape(DH // P, P, DIM).transpose(1, 0, 2)
    ).astype(BF16)
    c2c = np.ascontiguousarray(c2.reshape(DIM // P, P).T).astype(np.float32)

    # gathered relative-position bias, packed [128, H, 392]
    bias_full = bias_table[:, bias_idxs]      # [H, N, N]
    bias_pk = np.zeros((P, H, 2 * N), np.float32)
    bias_pk[:, :, 0:N] = bias_full[:, 0:P, :].transpose(1, 0, 2)
    bias_pk[:NT1, :, N:2 * N] = bias_full[:, P:N, :].transpose(1, 0, 2)
    bias_pk = bias_pk.astype(BF16)

    # x -> xT[dim, n] bf16, host-side: [B, N, DIM] -> [B, P, NKT, N]
    xt = np.ascontiguousarray(
        x.reshape(B, N, NKT, P).transpose(0, 3, 2, 1)
    ).astype(BF16)

    shared = {
        "wqk": wqk_l, "wv": wv_l, "wp": wp_l, "bias": bias_pk,
        "c1qk": c1qk, "c1v": c1v, "c2": c2c,
    }
    in_maps = []
    for c in range(NCORES):
        m = dict(shared)
        m["xt"] = np.ascontiguousarray(xt[c * BPC:(c + 1) * BPC])
        in_maps.append(m)
    return in_maps


def run_sharded(inputs, trace=False, **kwargs):
    from concourse.bass_utils import run_bass_kernel_spmd

    nc = _build_program()
    in_maps = _prepare_inputs(inputs)
    res = run_bass_kernel_spmd(
        nc, in_maps, list(range(NCORES)), trace=trace, **kwargs
    )
    y = np.concatenate([res.results[c]["y"] for c in range(NCORES)], axis=0)
    y = y.transpose(0, 3, 2, 1).reshape(B, N, DIM)
    return np.ascontiguousarray(y, dtype=np.float32), res


def kernel(**inputs) -> np.ndarray:
    y, _ = run_sharded(inputs, trace=False)
    return y


# revision 7
# speedup vs baseline: 1.1066x; 1.1066x over previous
"""Trainium2 Bass kernel for nn_Attention_17635135717804.

Dense transformer attention block (LeViT-style):
  qkv = BN(x @ Wqkv.T); per-head attention with gathered relative-position
  bias; softmax; o = attn @ v; y = BN(hardswish(o) @ Wproj.T).

Data-parallel over batch across 8 NeuronCores (16 batches/core). BN is
folded into weights host-side; softmax SCALE folded into Wq; x is
pre-transposed to xT[dim, n] bf16 on host (kills the on-device PE
transposes and their DVE evictions).

On real TRN2 the PE runs bf16 matmuls at ~0.17 ns/column (2x the cost
model), so this problem is bound by PSUM-eviction bandwidth on DVE+ACT,
not by the matmuls. The kernel is organized around that:
  - softmax normalize runs on DVE (bf16 2x mode) with reciprocals
    grouped 4 heads per op.
  - hardswish is 2 DVE passes (tensor_scalar + scalar_tensor_tensor)
    with the 1/6 folded into Wproj.
  - PSUM evictions are split across DVE and ACT to balance both queues
    (v + attn-transpose evicts on DVE; exp, qk and proj evicts on ACT;
    attn@v evicts split).
  - the batch loop is a 3-stage software pipeline: hardswish+proj of
    batch b-2 and qkv of batch b are emitted before attention of b-1,
    so the vector engines always have eviction work queued while the PE
    runs the next batch's matmuls.
"""

import numpy as np
import ml_dtypes

RES = 14
DIM = 512
KD = 64
H = 16
D = 256
DH = H * D            # 4096
HID = DH + 2 * H * KD  # 6144
B = 128
N = RES * RES         # 196
EPS = 1e-5
SCALE = KD ** -0.5

NCORES = 8
BPC = B // NCORES     # 16 batches per core
P = 128
NT1 = N - P           # 68: second token tile
NKT = DIM // P        # 4 k-tiles over input dim
QKF = 2 * H * KD      # 2048 qk features
BF16 = ml_dtypes.bfloat16

_PROGRAM_CACHE = {}


def _build_program(repeat=1):
    """Build the per-core Bass/Tile program (identical on all 8 cores).

    repeat>1 re-runs the whole batch loop (same data) for slope-based
    timing: T(R) - T(1) = (R-1) * kernel_time."""
    if repeat in _PROGRAM_CACHE:
        return _PROGRAM_CACHE[repeat]

    import concourse.bass as bass
    import concourse.mybir as mybir
    import concourse.tile as tile
    from concourse.masks import make_identity

    f32 = mybir.dt.float32
    bf16 = mybir.dt.bfloat16
    AF = mybir.ActivationFunctionType
    OP = mybir.AluOpType

    nc = bass.Bass("TRN2", target_bir_lowering=False, debug=False)

    xt_d = nc.dram_tensor("xt", [BPC, P, NKT, N], bf16, kind="ExternalInput").ap()
    wqk_d = nc.dram_tensor("wqk", [P, NKT, QKF], bf16, kind="ExternalInput").ap()
    wv_d = nc.dram_tensor("wv", [P, NKT, DH], bf16, kind="ExternalInput").ap()
    wp_d = nc.dram_tensor("wp", [P, DH // P, DIM], bf16, kind="ExternalInput").ap()
    bias_d = nc.dram_tensor("bias", [P, H, 2 * N], bf16, kind="ExternalInput").ap()
    c1qk_d = nc.dram_tensor("c1qk", [P, H], f32, kind="ExternalInput").ap()
    c1v_d = nc.dram_tensor("c1v", [P, DH // P], f32, kind="ExternalInput").ap()
    c2_d = nc.dram_tensor("c2", [P, DIM // P], f32, kind="ExternalInput").ap()
    y_d = nc.dram_tensor("y", [BPC, P, DIM // P, N], f32, kind="ExternalOutput").ap()

    from contextlib import ExitStack

    with tile.TileContext(nc) as tc:
        with ExitStack() as ctx:
            pool_ = lambda name, bufs, **kw: ctx.enter_context(
                tc.tile_pool(name=name, bufs=bufs, **kw)
            )
            singles = pool_("singles", 1)
            xTpool = pool_("xTpool", 3)
            qkpool = pool_("qkpool", 2)
            vpool = pool_("vpool", 2)
            epool = pool_("epool", 8)
            apool = pool_("apool", 3)
            sumpool = pool_("sumpool", 3)
            aTpool = pool_("aTpool", 4)
            zpool = pool_("zpool", 2)
            upool = pool_("upool", 2)
            yTpool = pool_("yTpool", 2)
            # PSUM: pmm (qk [128,196] / AV po [128,392]) + pv + ps + paT
            pmm = pool_("pmm", 2, space="PSUM")
            pv_pool = pool_("pv", 2, space="PSUM")
            ps_pool = pool_("ps", 2, space="PSUM")
            paT_pool = pool_("paT", 2, space="PSUM")

            # resident constants
            c1qk = singles.tile([P, H], f32)
            nc.scalar.dma_start(out=c1qk, in_=c1qk_d)
            c1v = singles.tile([P, DH // P], f32)
            nc.scalar.dma_start(out=c1v, in_=c1v_d)
            c2 = singles.tile([P, DIM // P], f32)
            nc.scalar.dma_start(out=c2, in_=c2_d)
            wqk = singles.tile([P, NKT, QKF], bf16)
            nc.scalar.dma_start(out=wqk, in_=wqk_d)
            wv = singles.tile([P, NKT, DH], bf16)
            for wc in range(4):
                nc.scalar.dma_start(
                    out=wv[:, :, wc * (DH // 4):(wc + 1) * (DH // 4)],
                    in_=wv_d[:, :, wc * (DH // 4):(wc + 1) * (DH // 4)],
                )
            bias = singles.tile([P, H, 2 * N], bf16)
            nc.scalar.dma_start(out=bias, in_=bias_d)
            wp = singles.tile([P, DH // P, DIM], bf16)
            nc.scalar.dma_start(out=wp, in_=wp_d)
            ident_b = singles.tile([P, P], bf16)
            make_identity(nc, ident_b)

            def load_xT(b):
                xT = xTpool.tile([P, NKT, N], bf16, tag="xT")
                nc.sync.dma_start(out=xT, in_=xt_d[b])
                return xT

            def emit_qk(xT):
                # qkT[2048, n] = Wqk @ xT, + c1qk bias, -> bf16 (ACT evict)
                qkT = qkpool.tile([P, H, N], bf16, tag="qk")
                for h in range(H):
                    pq = pmm.tile([P, N], f32, tag="mm")
                    for kt in range(NKT):
                        nc.tensor.matmul(
                            pq, wqk[:, kt, h * P:(h + 1) * P], xT[:, kt, :],
                            start=(kt == 0), stop=(kt == NKT - 1),
                        )
                    nc.scalar.activation(
                        out=qkT[:, h, :], in_=pq, func=AF.Identity,
                        bias=c1qk[:, h:h + 1], scale=1.0,
                    )
                return qkT

            def emit_v(xT):
                # v[n, 4096] = xT.T @ WvT (token-major; DVE evicts, no bias)
                v_sb = vpool.tile([P, 2, DH], bf16, tag="v")
                for mt in range(2):
                    rows = P if mt == 0 else NT1
                    for ntc in range(DH // 512):
                        pvt = pv_pool.tile([P, 512], f32, tag="pv")
                        for kt in range(NKT):
                            nc.tensor.matmul(
                                pvt[:rows],
                                xT[:, kt, mt * P:mt * P + rows],
                                wv[:, kt, ntc * 512:(ntc + 1) * 512],
                                start=(kt == 0), stop=(kt == NKT - 1),
                            )
                        nc.vector.tensor_copy(
                            out=v_sb[:rows, mt, ntc * 512:(ntc + 1) * 512],
                            in_=pvt[:rows],
                        )
                return v_sb

            def attn_scores(qkT, h, sums, sidx):
                # scores packed [128, 392]: bias preload (PE), qk matmuls,
                # exp+accum (ACT) -> e_sb bf16
                qo = (h % 2) * KD
                qt, kt_i = h // 2, 8 + h // 2
                s_ps = ps_pool.tile([P, 2 * N], f32, tag="ps")
                nc.tensor.matmul(
                    s_ps, ident_b, bias[:, h, :], start=True, stop=False,
                )
                nc.tensor.matmul(
                    s_ps[:, 0:N], qkT[qo:qo + KD, qt, 0:P],
                    qkT[qo:qo + KD, kt_i, :],
                    start=False, stop=False,
                )
                nc.tensor.matmul(
                    s_ps[:NT1, N:2 * N], qkT[qo:qo + KD, qt, P:N],
                    qkT[qo:qo + KD, kt_i, :],
                    start=False, stop=True,
                )
                e_sb = epool.tile([P, 2 * N], bf16, tag="e")
                nc.scalar.activation(
                    out=e_sb[:, 0:N], in_=s_ps[:, 0:N], func=AF.Exp,
                    accum_out=sums[:, 2 * sidx:2 * sidx + 1],
                )
                nc.scalar.activation(
                    out=e_sb[:NT1, N:2 * N], in_=s_ps[:NT1, N:2 * N],
                    func=AF.Exp, accum_out=sums[:NT1, 2 * sidx + 1:2 * sidx + 2],
                )
                return e_sb

            def attn_norm(e_sb, sums, sidx):
                # a = e * (1/rowsum), DVE TS with per-partition scalar
                a_sb = apool.tile([P, 2 * N], bf16, tag="a")
                nc.vector.tensor_scalar_mul(
                    out=a_sb[:, 0:N], in0=e_sb[:, 0:N],
                    scalar1=sums[:, 2 * sidx:2 * sidx + 1],
                )
                nc.vector.tensor_scalar_mul(
                    out=a_sb[:NT1, N:2 * N], in0=e_sb[:NT1, N:2 * N],
                    scalar1=sums[:NT1, 2 * sidx + 1:2 * sidx + 2],
                )
                return a_sb

            def attn_transpose(a_sb):
                paT = paT_pool.tile([P, 2 * N], bf16, tag="paT")
                nc.tensor.transpose(paT[:, 0:P], a_sb[:, 0:P], ident_b)
                nc.tensor.transpose(
                    paT[:, P:N], a_sb[:NT1, N:N + P], ident_b[:NT1, :NT1]
                )
                nc.tensor.transpose(paT[:NT1, N:N + P], a_sb[:, P:N], ident_b)
                nc.tensor.transpose(
                    paT[:NT1, N + P:2 * N], a_sb[:NT1, N + P:2 * N],
                    ident_b[:NT1, :NT1],
                )
                aT_sb = aTpool.tile([P, 2 * N], bf16, tag="aT")
                nc.vector.tensor_copy(out=aT_sb, in_=paT)
                return aT_sb

            def attn_av(h, aT_sb, v_sb, z_sb):
                # oT[d, n] = v.T @ attnT (+c1v at evict; evicts split DVE/ACT)
                for dt in range(2):
                    col = h * 2 + dt
                    po = pmm.tile([P, N], f32, tag="mm")
                    nc.tensor.matmul(
                        po, v_sb[:, 0, col * P:(col + 1) * P],
                        aT_sb[:, 0:N], start=True, stop=False,
                    )
                    nc.tensor.matmul(
                        po, v_sb[:NT1, 1, col * P:(col + 1) * P],
                        aT_sb[:NT1, N:2 * N], start=False, stop=True,
                    )
                    if col % 3 == 0:   # ~1/3 on DVE, 2/3 on ACT
                        nc.vector.tensor_scalar_add(
                            out=z_sb[:, col, :], in0=po,
                            scalar1=c1v[:, col:col + 1],
                        )
                    else:
                        nc.scalar.activation(
                            out=z_sb[:, col, :], in_=po, func=AF.Identity,
                            bias=c1v[:, col:col + 1], scale=1.0,
                        )

            def emit_attention(qkT, v_sb, z_hsw):
                # 16 heads, software-pipelined:
                #   scores/exp(h) | norm+transpose(h-4) | AV(h-6);
                # hardswish chunks of the PREVIOUS batch's z are
                # interleaved so DVE/PE never starve at step start.
                z_sb = zpool.tile([P, DH // P, N], bf16, tag="z")
                e_q, aT_q = {}, {}
                sums_g = {}
                for h in range(H + 6):
                    if h < H:
                        g = h // 4
                        if h % 4 == 0:
                            sums_g[g] = sumpool.tile([P, 8], f32, tag="sums", name=f"sums{g}")
                        e_q[h] = attn_scores(qkT, h, sums_g[g], h % 4)
                        if h % 4 == 3:
                            nc.vector.reciprocal(out=sums_g[g], in_=sums_g[g])
                    if 4 <= h < H + 4:
                        hh = h - 4
                        a_sb = attn_norm(e_q.pop(hh), sums_g[hh // 4], hh % 4)
                        aT_q[hh] = attn_transpose(a_sb)
                    if h >= 6:
                        hh = h - 6
                        attn_av(hh, aT_q.pop(hh), v_sb, z_sb)
                        if z_hsw is not None and hh % 4 == 3:
                            emit_hardswish_chunk(z_hsw, (hh // 4) * 8)
                return z_sb

            def emit_hardswish_chunk(z_sb, c0):
                # hardswish*6 (1/6 folded into Wp): u = relu(z+3) [DVE TS],
                # z = min(u, 6) * z [DVE STT]
                CH = 8
                u = upool.tile([P, CH, N], bf16, tag="u")
                zc = z_sb[:, c0:c0 + CH, :]
                nc.vector.tensor_scalar(
                    out=u, in0=zc, scalar1=3.0, scalar2=0.0,
                    op0=OP.add, op1=OP.max,
                )
                nc.vector.scalar_tensor_tensor(
                    out=zc, in0=u, scalar=6.0, in1=zc,
                    op0=OP.min, op1=OP.mult,
                )

            def emit_proj(z_sb, b):
                # yT[512, n] = (Wp/6) @ hardswish6(oT), + c2 (ACT evict)
                yT = yTpool.tile([P, DIM // P, N], f32, tag="yT")
                for mt in range(DIM // P):
                    py = pmm.tile([P, N], f32, tag="mm")
                    for kt in range(DH // P):
                        nc.tensor.matmul(
                            py, wp[:, kt, mt * P:(mt + 1) * P], z_sb[:, kt, :],
                            start=(kt == 0), stop=(kt == DH // P - 1),
                        )
                    nc.scalar.activation(
                        out=yT[:, mt, :], in_=py, func=AF.Identity,
                        bias=c2[:, mt:mt + 1], scale=1.0,
                    )
                nc.sync.dma_start(out=y_d[b], in_=yT)

            # ---- 3-stage pipelined batch loop ----
            # step i: attention(b-1) [hardswish(b-2) interleaved] ->
            #         proj(b-2) -> qk(b) -> v(b)
            batch_seq = list(range(BPC)) * repeat
            nb = len(batch_seq)
            xT_cur = load_xT(batch_seq[0])
            xT_next = load_xT(batch_seq[1]) if nb > 1 else None
            qkT_prev = v_prev = None     # stage of batch i-1
            z_prev2 = None               # z of batch i-2
            b_prev2 = None
            for i in range(nb + 2):
                z_new = None
                if qkT_prev is not None:
                    z_new = emit_attention(qkT_prev, v_prev, z_prev2)
                elif z_prev2 is not None:
                    for c0 in range(0, DH // P, 8):
                        emit_hardswish_chunk(z_prev2, c0)
                if z_prev2 is not None:
                    emit_proj(z_prev2, b_prev2)
                if i < nb:
                    qkT_cur = emit_qk(xT_cur)
                    v_cur = emit_v(xT_cur)
                    xT_cur = xT_next
                    xT_next = load_xT(batch_seq[i + 2]) if i + 2 < nb else None
                else:
                    qkT_cur = v_cur = None
                z_prev2 = z_new
                b_prev2 = batch_seq[i - 1] if 1 <= i <= nb else None
                qkT_prev, v_prev = qkT_cur, v_cur

    _split_matmul_waits(nc, mybir)
    _PROGRAM_CACHE[repeat] = nc
    return nc


def _split_matmul_waits(nc, mybir):
    """Walrus's per-instruction ISA structs accept only one sync wait;
    hoist extra waits onto injected single-wait NoOps on the same engine."""
    multiwait_ok = ("InstCall",)
    nid = [0]
    for f in nc.m.functions:
        for blk in f.blocks:
            insts = blk.instructions
            out = []
            changed = False
            for i in insts:
                si = i.sync_info
                if (
                    type(i).__name__ not in multiwait_ok
                    and si is not None
                    and si.on_wait
                    and len(si.on_wait) > 1
                ):
                    for w in si.on_wait[1:]:
                        nop = mybir.InstNoOp(
                            name=f"waitnop-{nid[0]}", ins=[], outs=[]
                        )
                        nid[0] += 1
                        nop.engine = i.engine
                        nop.sync_info = mybir.SyncInfo(
                            on_wait=[w], on_update=[]
                        )
                        out.append(nop)
                    i.sync_info = mybir.SyncInfo(
                        on_wait=[si.on_wait[0]],
                        on_update=list(si.on_update or []),
                    )
                    changed = True
                out.append(i)
            if changed:
                blk.instructions = out


def _prepare_inputs(inputs):
    """Fold BN into weights, reorder layouts, gather bias, transpose x;
    build per-core input maps."""
    f = lambda k: np.asarray(inputs[k], dtype=np.float32)
    x = f("x")
    w_qkv = f("w_qkv")
    g1, b1, m1, v1 = f("g1"), f("b1"), f("m1"), f("v1")
    bias_table = f("bias_table")
    w_proj = f("w_proj")
    g2, b2, m2, v2 = f("g2"), f("b2"), f("m2"), f("v2")
    bias_idxs = np.asarray(inputs["bias_idxs"])

    s1 = g1 / np.sqrt(v1 + EPS)
    c1 = b1 - m1 * s1
    W1 = w_qkv * s1[:, None]          # [HID, DIM]
    W1h = W1.reshape(H, 2 * KD + D, DIM)
    c1h = c1.reshape(H, 2 * KD + D)

    # qk features: tiles 0..7 hold q of head-pairs (pre-scaled by SCALE),
    # tiles 8..15 hold k of head-pairs; head h sits at partition (h%2)*64
    # of tile h//2 (q) and tile 8+h//2 (k) so q/k share a base partition.
    wqk_feat = np.empty((QKF, DIM), np.float32)
    c1qk = np.empty((P, H), np.float32)
    for h in range(H):
        qrow = (h // 2) * P + (h % 2) * KD
        krow = 8 * P + qrow
        wqk_feat[qrow:qrow + KD] = W1h[h, :KD] * SCALE
        wqk_feat[krow:krow + KD] = W1h[h, KD:2 * KD]
        c1qk[(h % 2) * KD:(h % 2) * KD + KD, h // 2] = c1h[h, :KD] * SCALE
        c1qk[(h % 2) * KD:(h % 2) * KD + KD, 8 + h // 2] = c1h[h, KD:2 * KD]
    # lhsT layout [dim_p, ktile, feat]
    wqk_l = np.ascontiguousarray(
        wqk_feat.T.reshape(NKT, P, QKF).transpose(1, 0, 2)
    ).astype(BF16)

    # v features (h, d) -> rhs layout [dim_p, ktile, dh]
    wv_feat = W1h[:, 2 * KD:, :].reshape(DH, DIM)
    wv_l = np.ascontiguousarray(
        wv_feat.T.reshape(NKT, P, DH).transpose(1, 0, 2)
    ).astype(BF16)
    c1v = np.ascontiguousarray(
        c1h[:, 2 * KD:].reshape(DH).reshape(DH // P, P).T
    ).astype(np.float32)

    s2 = g2 / np.sqrt(v2 + EPS)
    c2 = b2 - m2 * s2
    # hardswish computed as z*clip(z+3,0,6); fold the /6 into Wproj
    W2 = w_proj * s2[:, None] * (1.0 / 6.0)   # [DIM, DH]
    wp_l = np.ascontiguousarray(
        W2.T.reshape(DH // P, P, DIM).transpose(1, 0, 2)
    ).astype(BF16)
    c2c = np.ascontiguousarray(c2.reshape(DIM // P, P).T).astype(np.float32)

    # gathered relative-position bias, packed [128, H, 392]
    bias_full = bias_table[:, bias_idxs]      # [H, N, N]
    bias_pk = np.zeros((P, H, 2 * N), np.float32)
    bias_pk[:, :, 0:N] = bias_full[:, 0:P, :].transpose(1, 0, 2)
    bias_pk[:NT1, :, N:2 * N] = bias_full[:, P:N, :].transpose(1, 0, 2)
    bias_pk = bias_pk.astype(BF16)

    # x -> xT[dim, n] bf16, host-side: [B, N, DIM] -> [B, P, NKT, N]
    xt = np.ascontiguousarray(
        x.reshape(B, N, NKT, P).transpose(0, 3, 2, 1)
    ).astype(BF16)

    shared = {
        "wqk": wqk_l, "wv": wv_l, "wp": wp_l, "bias": bias_pk,
        "c1qk": c1qk, "c1v": c1v, "c2": c2c,
    }
    in_maps = []
    for c in range(NCORES):
        m = dict(shared)
        m["xt"] = np.ascontiguousarray(xt[c * BPC:(c + 1) * BPC])
        in_maps.append(m)
    return in_maps


def run_sharded(inputs, trace=False, **kwargs):
    from concourse.bass_utils import run_bass_kernel_spmd

    nc = _build_program()
    in_maps = _prepare_inputs(inputs)
    res = run_bass_kernel_spmd(
        nc, in_maps, list(range(NCORES)), trace=trace, **kwargs
    )
    y = np.concatenate([res.results[c]["y"] for c in range(NCORES)], axis=0)
    y = y.transpose(0, 3, 2, 1).reshape(B, N, DIM)
    return np.ascontiguousarray(y, dtype=np.float32), res


def kernel(**inputs) -> np.ndarray:
    y, _ = run_sharded(inputs, trace=False)
    return y
